# revision 33
# baseline (speedup 1.0000x reference)
"""Trainium2 Bass kernel for DeiT-style attention + depthwise-conv block.

Computes, for x [N=32, L=577, C=768]:
  qkv = x @ w_qkv.T -> q,k,v (12 heads, hd=64)
  attn = softmax(q k^T / 8) @ v
  out  = attn (+ depthwise3x3(v) on patch tokens) @ w_proj.T + b_proj

Sharding: data-parallel over batch, 4 samples per core x 8 NeuronCores.

Structure (per core): the attention inner loop is ACT(exp)-paced, so PE
work from the NEXT sample (qkv/v65 matmuls) and the PREVIOUS sample
(projection) is interleaved into the per-head gaps between the scores
and PV matmuls.  Scores matmuls write bf16 PSUM (one bank, single
N=577 matmul).  Softmax normalization is a PSUM-read divide (no
reciprocal), conv is 9 fused multiply-add ops on persistent zero-
bordered pad buffers.
"""
import sys

sys.path.insert(0, "/opt/trn_rl_repo")

import numpy as np

import concourse.bacc as bacc
import concourse.mybir as mybir
import concourse.tile as tile
from concourse.bass_utils import run_bass_kernel_spmd

F32 = mybir.dt.float32
F32R = mybir.dt.float32r
BF16 = mybir.dt.bfloat16
Exp = mybir.ActivationFunctionType.Exp
MULT = mybir.AluOpType.mult
ADD = mybir.AluOpType.add
DIV = mybir.AluOpType.divide

N_CORES = 8
S = 4            # samples per core
C, L, H, HD = 768, 577, 12, 64
CT = C // 128    # 6 channel tiles
KT = 3 * C // 128  # 18 qkv row tiles
SCALE = HD ** -0.5
L_CHUNKS = [(i * 128, min(128, L - i * 128)) for i in range((L + 127) // 128)]
NCH = len(L_CHUNKS)  # 5
IMG = 24         # spatial side; L-1 == IMG*IMG
PAD = IMG + 2    # padded side

_CACHE = {}
last_results = None  # BassKernelResults of the most recent run (for test harness)

import os
# debug bisect toggles (default = all optimizations on)
OPT_POOLCONV = os.environ.get("KOPT_POOLCONV", "1") == "1"
OPT_WDMA_POOL = os.environ.get("KOPT_WDMA_POOL", "1") == "1"
OPT_PAIR = os.environ.get("KOPT_PAIR", "1") == "1"


def _build_nc(repeat=1, stages="full"):
    key = (repeat, stages)
    if key in _CACHE:
        return _CACHE[key]
    nc = bacc.Bacc("TRN2", target_bir_lowering=False, debug=False,
                   num_devices=N_CORES)
    xT_d = nc.declare_dram_parameter("xT", [S, C, L], BF16, isOutput=False)
    wqkvT_d = nc.declare_dram_parameter("wqkvT", [C, 3 * C], BF16, isOutput=False)
    wprojT_d = nc.declare_dram_parameter("wprojT", [C, C], BF16, isOutput=False)
    wdwc_d = nc.declare_dram_parameter("wdwc", [C, 9], F32, isOutput=False)
    bdwc_d = nc.declare_dram_parameter("bdwc", [C, 1], F32, isOutput=False)
    bproj_d = nc.declare_dram_parameter("bproj", [1, C], F32, isOutput=False)
    y_d = nc.declare_dram_parameter("y", [S, L, C], F32, isOutput=True)

    with tile.TileContext(nc) as tc:
        with tc.tile_pool(name="wpool", bufs=1) as wpool, \
             tc.tile_pool(name="work", bufs=2) as work, \
             tc.tile_pool(name="mm", bufs=3, space="PSUM") as psum_mm, \
             tc.tile_pool(name="pv", bufs=1, space="PSUM") as psum_pv:

            # ---- resident weights (loaded once; q parts first so the
            # first qkv matmuls can start before the rest arrives) ----
            wqkv = []
            for k in range(CT):
                t = wpool.tile([128, 3 * C], BF16, tag="wqkv", bufs=CT,
                               name=f"wqkv{k}")
                wqkv.append(t)
            # single-shot build: prefetch sample-0 x ahead of the weight
            # loads so the first qkv matmuls can start as early as possible
            pre_x0 = None
            if repeat == 1:
                pre_x0 = []
                for k in range(CT):
                    t = work.tile([128, L], BF16, tag="xT", bufs=2 * CT,
                                  name=f"xT0p{k}")
                    nc.sync.dma_start(t[:], xT_d[0, k * 128:(k + 1) * 128, :])
                    pre_x0.append(t)

            # weight loads go out on the gpsimd DMA queue so they overlap
            # with the sample-0 xT loads issued on the sync queue
            wdma = nc.gpsimd if OPT_WDMA_POOL else nc.sync
            for part in range(3):
                for k in range(CT):
                    wdma.dma_start(
                        wqkv[k][:, part * C:(part + 1) * C],
                        wqkvT_d[k * 128:(k + 1) * 128, part * C:(part + 1) * C])
            wprojT = []
            for k in range(CT):
                t = wpool.tile([128, C], BF16, tag="wprojT", bufs=CT,
                               name=f"wprojT{k}")
                wdma.dma_start(t[:], wprojT_d[k * 128:(k + 1) * 128, :])
                wprojT.append(t)
            wdwc = []
            bdwc = []
            for k in range(CT):
                t = wpool.tile([128, 9], F32, tag="wdwc", bufs=CT, name=f"wdwc{k}")
                wdma.dma_start(t[:], wdwc_d[k * 128:(k + 1) * 128, :])
                wdwc.append(t)
                t = wpool.tile([128, 1], F32, tag="bdwc", bufs=CT, name=f"bdwc{k}")
                wdma.dma_start(t[:], bdwc_d[k * 128:(k + 1) * 128, :])
                bdwc.append(t)
            bproj_row = wpool.tile([1, C], F32, tag="bprow")
            wdma.dma_start(bproj_row[:], bproj_d[:])
            bproj_bc = wpool.tile([128, C], F32, tag="bpbc")
            nc.gpsimd.partition_broadcast(bproj_bc[:], bproj_row[:])

            # persistent zero-bordered conv pad buffers (interior rewritten
            # per use; borders stay zero)
            vpad = []
            for i in range(2):
                t = wpool.tile([128, PAD * PAD], BF16, tag="vpad", bufs=2,
                               name=f"vpad{i}")
                nc.vector.memset(t[:], 0.0)
                vpad.append(t)
            # persistent v65 tiles (two sets of NCH); ones column written once
            v65_all = []
            for i in range(2 * NCH):
                t = wpool.tile([128, H * 65], BF16, tag="v65", bufs=2 * NCH,
                               name=f"v65_{i}")
                t3 = t[:].rearrange("p (h w) -> p h w", h=H, w=65)
                nc.vector.memset(t3[:, :, 64:65], 1.0)
                v65_all.append(t)

            import contextlib
            rep_ctx = tc.For_i(0, repeat, 1) if repeat > 1 else contextlib.nullcontext()
            with rep_ctx:
                state = {}
                evict_ctr = [0]
                evict_act_period = [2]  # 1-in-N evictions go to ACT

                def evict(dst_ap, src_ap):
                    # split PSUM->SBUF evictions between ACT and DVE
                    if evict_ctr[0] % evict_act_period[0] == 0:
                        nc.scalar.copy(dst_ap, src_ap)
                    else:
                        nc.vector.tensor_copy(dst_ap, src_ap)
                    evict_ctr[0] += 1

                def mm_tile():
                    return psum_mm.tile([128, 768], F32, tag="mm", name="mmp")

                def emit_sample_inputs(s):
                    st = {"xT": [], "qk": [], "vch": [],
                          "v65": [v65_all[(s % 2) * NCH + ci] for ci in range(NCH)]}
                    if s == 0 and pre_x0 is not None:
                        st["xT"] = pre_x0
                        state[s] = st
                        return st
                    for k in range(CT):
                        t = work.tile([128, L], BF16, tag="xT", bufs=2 * CT,
                                      name=f"xT{k}")
                        nc.sync.dma_start(t[:], xT_d[s, k * 128:(k + 1) * 128, :])
                        st["xT"].append(t)
                    state[s] = st
                    return st

                def qkv_mtile(s, m):
                    st = state[s]
                    p = mm_tile()
                    for k in range(CT):
                        w_ap = wqkv[k][:, m * 128:(m + 1) * 128]
                        for (n0, nn) in ((0, 512), (512, 65)):
                            nc.tensor.matmul(
                                p[:, n0:n0 + nn], w_ap,
                                st["xT"][k][:, n0:n0 + nn],
                                start=(k == 0), stop=(k == CT - 1))
                    dst = work.tile([128, L], BF16,
                                    tag="qk" if m < 12 else "vch",
                                    bufs=24 if m < 12 else 2 * CT,
                                    name=f"qkv{m}")
                    evict(dst[:], p[:, 0:L])
                    (st["qk"] if m < 12 else st["vch"]).append(dst)

                def v65_chunk(s, ci):
                    st = state[s]
                    (l0, lp) = L_CHUNKS[ci]
                    t = st["v65"][ci]
                    t3 = t[:].rearrange("p (h w) -> p h w", h=H, w=65)
                    p = mm_tile()
                    for (n0, nn) in ((0, 512), (512, 256)):
                        for k in range(CT):
                            nc.tensor.matmul(
                                p[0:lp, n0:n0 + nn],
                                st["xT"][k][:, l0:l0 + lp],
                                wqkv[k][:, 2 * C + n0:2 * C + n0 + nn],
                                start=(k == 0), stop=(k == CT - 1))
                    evict(t3[0:lp, :, 0:64],
                          p[0:lp, 0:768].rearrange("p (h w) -> p h w", h=H, w=64))

                def make_fill_thunks(s):
                    return ([lambda m=m: qkv_mtile(s, m) for m in range(KT)]
                            + [lambda ci=ci: v65_chunk(s, ci) for ci in range(NCH)])

                def proj_chunk(s, ci):
                    st = state[s]
                    (l0, lp) = L_CHUNKS[ci]
                    attn = st["attn"]
                    p = mm_tile()
                    for (n0, nn) in ((0, 512), (512, 256)):
                        for k in range(CT):
                            nc.tensor.matmul(
                                p[0:lp, n0:n0 + nn],
                                attn[k][:, l0:l0 + lp],
                                wprojT[k][:, n0:n0 + nn],
                                start=(k == 0), stop=(k == CT - 1))
                    ysb = work.tile([128, C], F32, tag="ysb", bufs=2)
                    nc.vector.tensor_tensor(
                        out=ysb[0:lp, :], in0=p[0:lp, :], in1=bproj_bc[0:lp, :],
                        op=ADD)
                    nc.sync.dma_start(y_d[s, l0:l0 + lp, :], ysb[0:lp, :])

                def make_proj_thunks(s):
                    return [lambda ci=ci: proj_chunk(s, ci) for ci in range(NCH)]

                def scores_pair(s, hp):
                    # heads 2hp (rows 0:64) and 2hp+1 (rows 64:128) emitted
                    # chunk-interleaved: adjacent matmuls hit disjoint PE row
                    # groups and run concurrently on hardware
                    st = state[s]
                    qt = st["qk"][hp]
                    kt_ = st["qk"][6 + hp]
                    expA, expB = [], []
                    if OPT_PAIR:
                        order = [(l0, lp, hb, e) for (l0, lp) in L_CHUNKS
                                 for hb, e in ((0, expA), (64, expB))]
                    else:
                        order = ([(l0, lp, 0, expA) for (l0, lp) in L_CHUNKS]
                                 + [(l0, lp, 64, expB) for (l0, lp) in L_CHUNKS])
                    for (l0, lp, hb, exps) in order:
                        if True:
                            p = mm_tile()
                            for (n0, nn) in ((0, 512), (512, 65)):
                                nc.tensor.matmul(p[0:lp, n0:n0 + nn],
                                                 kt_[hb:hb + 64, l0:l0 + lp],
                                                 qt[hb:hb + 64, n0:n0 + nn],
                                                 start=True, stop=True)
                            e = work.tile([128, L], BF16, tag="expS", bufs=12,
                                          name="expSt")
                            nc.scalar.activation(e[0:lp, :], p[0:lp, 0:L], Exp,
                                                 scale=SCALE)
                            exps.append(e)
                    return expA, expB

                def pv_head(s, h, expS):
                    st = state[s]
                    pv = psum_pv.tile([128, L], F32, tag="pv")
                    for ci, (l0, lp) in enumerate(L_CHUNKS):
                        for (n0, nn) in ((0, 512), (512, 65)):
                            nc.tensor.matmul(
                                pv[0:65, n0:n0 + nn],
                                st["v65"][ci][0:lp, h * 65:(h + 1) * 65],
                                expS[ci][0:lp, n0:n0 + nn],
                                start=(ci == 0), stop=(ci == NCH - 1))
                    # NOTE: reciprocal_approx_fast reading PSUM directly
                    # returns garbage on hardware -- stage sums via SBUF
                    sums = work.tile([1, L], F32, tag="sums", bufs=2,
                                     name="sums")
                    nc.scalar.copy(sums[:], pv[64:65, :])
                    rec = work.tile([1, L], F32, tag="rec", bufs=2, name="rec")
                    nc.vector.reciprocal_approx_fast(out=rec[:], in_=sums[:])
                    bc = work.tile([64, L], F32, tag="bc", bufs=2, name="bc")
                    nc.gpsimd.partition_broadcast(bc[:], rec[:])
                    hb = (h % 2) * 64
                    nc.vector.tensor_tensor(
                        out=st["attn"][h // 2][hb:hb + 64, :],
                        in0=pv[0:64, :], in1=bc[:], op=MULT)

                def conv_prep(s, ct):
                    st = state[s]
                    vp = vpad[ct % 2]
                    vp3 = vp[:].rearrange("p (y x) -> p y x", y=PAD, x=PAD)
                    conv_eng = nc.gpsimd if OPT_POOLCONV else nc.vector
                    conv_eng.tensor_copy(
                        vp3[:, 1:1 + IMG, 1:1 + IMG],
                        st["vch"][ct][:, 1:L].rearrange("p (y x) -> p y x",
                                                        y=IMG, x=IMG))
                    acc = work.tile([128, IMG * IMG], BF16, tag="cacc", bufs=2,
                                    name="cacc")
                    acc3 = acc[:].rearrange("p (y x) -> p y x", y=IMG, x=IMG)

                    def tap(t):
                        return vp3[:, t // 3:t // 3 + IMG, t % 3:t % 3 + IMG]

                    nc.vector.tensor_scalar(
                        out=acc3, in0=tap(4), scalar1=wdwc[ct][:, 4:5],
                        scalar2=None, op0=MULT)
                    for i, t in enumerate([0, 1, 2, 3, 5, 6, 7, 8]):
                        tmp = work.tile([128, IMG * IMG], BF16, tag="ctmp",
                                        bufs=6, name="ctmp")
                        tmp3 = tmp[:].rearrange("p (y x) -> p y x", y=IMG, x=IMG)
                        nc.vector.tensor_scalar(
                            out=tmp3, in0=tap(t), scalar1=wdwc[ct][:, t:t + 1],
                            scalar2=None, op0=MULT)
                        # gpsimd absorbs part of the accumulation (DVE is the
                        # busier engine); it gets its own partial sum so the
                        # two engines' chains stay disjoint.
                        if OPT_POOLCONV and i == 2:
                            pacc = work.tile([128, IMG * IMG], BF16, tag="pacc",
                                             bufs=2, name="pacc")
                            nc.gpsimd.tensor_copy(pacc[:], tmp[:])
                        elif OPT_POOLCONV and i > 2:
                            nc.gpsimd.tensor_tensor(out=pacc[:], in0=pacc[:],
                                                    in1=tmp[:], op=ADD)
                        else:
                            nc.vector.tensor_tensor(out=acc[:], in0=acc[:],
                                                    in1=tmp[:], op=ADD)
                    if OPT_POOLCONV:
                        nc.vector.tensor_tensor(out=acc[:], in0=acc[:],
                                                in1=pacc[:], op=ADD)
                    return acc

                def conv_add(s, ct, acc):
                    # attn[:, 1:] += acc + b_dwc
                    nc.vector.scalar_tensor_tensor(
                        out=state[s]["attn"][ct][:, 1:L], in0=acc[:],
                        scalar=bdwc[ct][:, 0:1],
                        in1=state[s]["attn"][ct][:, 1:L],
                        op0=ADD, op1=ADD)

                # ---- prologue: sample 0 inputs + qkv/v65 emitted directly
                # (mm/sc pool alternation -- nothing else needs sc yet) ----
                emit_sample_inputs(0)
                for t in make_fill_thunks(0):
                    t()

                for s in range(S):
                    st = state[s]
                    st["attn"] = [work.tile([128, L], BF16, tag="attn", bufs=18,
                                            name=f"attn{ct}") for ct in range(CT)]
                    fillers = []
                    if s + 1 < S:
                        emit_sample_inputs(s + 1)
                        fillers += make_fill_thunks(s + 1)
                    # projections trail by two samples so the tail sample's
                    # exp-waits still have PE filler work
                    if s - 2 >= 0:
                        fillers += make_proj_thunks(s - 2)
                    if s == S - 1:
                        fillers += make_proj_thunks(s - 1)

                    if stages == "qkv":
                        zsrc = work.tile([128, L], F32, tag="zsrc", bufs=1,
                                         name="zsrc")
                        nc.vector.memset(zsrc[:], 0.0)
                        for ct in range(CT):
                            nc.vector.tensor_copy(st["attn"][ct][:], zsrc[:])
                        for t in fillers:
                            t()
                        continue

                    # ACT paces the attention inner loop (5 exp per head);
                    # keep it mostly exp there
                    evict_act_period[0] = 4
                    nf = len(fillers)
                    done = 0
                    for hp in range(H // 2):
                        expA, expB = scores_pair(s, hp)
                        if stages == "full":
                            acc = conv_prep(s, hp)
                        # PE fillers between scores and PV cover the exp wait
                        target = ((2 * hp + 1) * nf) // H
                        while done < target:
                            fillers[done]()
                            done += 1
                        pv_head(s, 2 * hp, expA)
                        target = ((2 * hp + 2) * nf) // H
                        while done < target:
                            fillers[done]()
                            done += 1
                        pv_head(s, 2 * hp + 1, expB)
                        if stages == "full":
                            conv_add(s, hp, acc)
                    evict_act_period[0] = 2

                # final projection (no attention loop left to hide it in)
                for t in make_proj_thunks(S - 1):
                    t()

    nc.compile()
    _CACHE[key] = nc
    return nc


def make_in_maps(x, w_qkv, w_proj, b_proj, w_dwc, b_dwc):
    x = np.asarray(x, dtype=np.float32)
    N = x.shape[0]
    assert N == N_CORES * S
    import ml_dtypes
    wqkvT = np.ascontiguousarray(
        np.asarray(w_qkv, np.float32).T.astype(ml_dtypes.bfloat16))    # [C, 3C]
    wprojT = np.ascontiguousarray(
        np.asarray(w_proj, np.float32).T.astype(ml_dtypes.bfloat16))   # [C, C]
    wdwc9 = np.ascontiguousarray(np.asarray(w_dwc, np.float32).reshape(C, 9))
    bdwc = np.ascontiguousarray(np.asarray(b_dwc, np.float32).reshape(C, 1))
    bproj = np.ascontiguousarray(np.asarray(b_proj, np.float32).reshape(1, C))

    in_maps = []
    for i in range(N_CORES):
        xs = x[i * S:(i + 1) * S]                       # [S, L, C]
        xT = np.ascontiguousarray(
            xs.transpose(0, 2, 1).astype(ml_dtypes.bfloat16))  # [S, C, L]
        in_maps.append({"xT": xT, "wqkvT": wqkvT, "wprojT": wprojT,
                        "wdwc": wdwc9, "bdwc": bdwc, "bproj": bproj})
    return in_maps


def kernel(x, w_qkv, w_proj, b_proj, w_dwc, b_dwc):
    global last_results
    nc = _build_nc()
    in_maps = make_in_maps(x, w_qkv, w_proj, b_proj, w_dwc, b_dwc)
    last_results = run_bass_kernel_spmd(nc, in_maps, list(range(N_CORES)))
    y = np.concatenate([r["y"] for r in last_results.results], axis=0)
    return y.astype(np.float32)


# revision 38
# speedup vs baseline: 1.1215x; 1.1215x over previous
"""Trainium2 Bass kernel for DeiT-style attention + depthwise-conv block.

Computes, for x [N=32, L=577, C=768]:
  qkv = x @ w_qkv.T -> q,k,v (12 heads, hd=64)
  attn = softmax(q k^T / 8) @ v
  out  = attn (+ depthwise3x3(v) on patch tokens) @ w_proj.T + b_proj

Sharding: data-parallel over batch, 4 samples per core x 8 NeuronCores.

Structure (per core): the attention inner loop is ACT(exp)-paced, so PE
work from the NEXT sample (qkv/v65 matmuls) and the PREVIOUS sample
(projection) is interleaved into the per-head gaps between the scores
and PV matmuls.  Scores matmuls write bf16 PSUM (one bank, single
N=577 matmul).  Softmax normalization is a PSUM-read divide (no
reciprocal), conv is 9 fused multiply-add ops on persistent zero-
bordered pad buffers.
"""
import sys

sys.path.insert(0, "/opt/trn_rl_repo")

import numpy as np

import concourse.bacc as bacc
import concourse.mybir as mybir
import concourse.tile as tile
from concourse.bass_utils import run_bass_kernel_spmd

F32 = mybir.dt.float32
F32R = mybir.dt.float32r
BF16 = mybir.dt.bfloat16
Exp = mybir.ActivationFunctionType.Exp
MULT = mybir.AluOpType.mult
ADD = mybir.AluOpType.add
DIV = mybir.AluOpType.divide

N_CORES = 8
S = 4            # samples per core
C, L, H, HD = 768, 577, 12, 64
CT = C // 128    # 6 channel tiles
KT = 3 * C // 128  # 18 qkv row tiles
SCALE = HD ** -0.5
L_CHUNKS = [(i * 128, min(128, L - i * 128)) for i in range((L + 127) // 128)]
NCH = len(L_CHUNKS)  # 5
IMG = 24         # spatial side; L-1 == IMG*IMG
PAD = IMG + 2    # padded side

_CACHE = {}
last_results = None  # BassKernelResults of the most recent run (for test harness)

import os
# debug bisect toggles (default = all optimizations on)
OPT_POOLCONV = os.environ.get("KOPT_POOLCONV", "1") == "1"
OPT_WDMA_POOL = os.environ.get("KOPT_WDMA_POOL", "1") == "1"
OPT_PAIR = os.environ.get("KOPT_PAIR", "1") == "1"


def _build_nc(repeat=1, stages="full"):
    key = (repeat, stages)
    if key in _CACHE:
        return _CACHE[key]
    nc = bacc.Bacc("TRN2", target_bir_lowering=False, debug=False,
                   num_devices=N_CORES)
    xT_d = nc.declare_dram_parameter("xT", [S, C, L], BF16, isOutput=False)
    wqkvT_d = nc.declare_dram_parameter("wqkvT", [C, 3 * C], BF16, isOutput=False)
    wprojT_d = nc.declare_dram_parameter("wprojT", [C, C], BF16, isOutput=False)
    wdwc_d = nc.declare_dram_parameter("wdwc", [C, 9], F32, isOutput=False)
    bdwc_d = nc.declare_dram_parameter("bdwc", [C, 1], F32, isOutput=False)
    bproj_d = nc.declare_dram_parameter("bproj", [1, C], F32, isOutput=False)
    y_d = nc.declare_dram_parameter("y", [S, L, C], F32, isOutput=True)

    with tile.TileContext(nc) as tc:
        with tc.tile_pool(name="wpool", bufs=1) as wpool, \
             tc.tile_pool(name="work", bufs=2) as work, \
             tc.tile_pool(name="mm", bufs=1, space="PSUM") as psum_mm, \
             tc.tile_pool(name="sc", bufs=2, space="PSUM") as psum_sc, \
             tc.tile_pool(name="pv", bufs=1, space="PSUM") as psum_pv:

            # ---- resident weights (loaded once; q parts first so the
            # first qkv matmuls can start before the rest arrives) ----
            wqkv = []
            for k in range(CT):
                t = wpool.tile([128, 3 * C], BF16, tag="wqkv", bufs=CT,
                               name=f"wqkv{k}")
                wqkv.append(t)
            # single-shot build: prefetch sample-0 x ahead of the weight
            # loads so the first qkv matmuls can start as early as possible
            pre_x0 = None
            if repeat == 1:
                pre_x0 = []
                for k in range(CT):
                    t = work.tile([128, L], BF16, tag="xT", bufs=2 * CT,
                                  name=f"xT0p{k}")
                    nc.sync.dma_start(t[:], xT_d[0, k * 128:(k + 1) * 128, :])
                    pre_x0.append(t)

            # weight loads go out on the gpsimd DMA queue so they overlap
            # with the sample-0 xT loads issued on the sync queue
            wdma = nc.gpsimd if OPT_WDMA_POOL else nc.sync
            for part in range(3):
                for k in range(CT):
                    wdma.dma_start(
                        wqkv[k][:, part * C:(part + 1) * C],
                        wqkvT_d[k * 128:(k + 1) * 128, part * C:(part + 1) * C])
            wprojT = []
            for k in range(CT):
                t = wpool.tile([128, C], BF16, tag="wprojT", bufs=CT,
                               name=f"wprojT{k}")
                wdma.dma_start(t[:], wprojT_d[k * 128:(k + 1) * 128, :])
                wprojT.append(t)
            wdwc = []
            bdwc = []
            for k in range(CT):
                t = wpool.tile([128, 9], F32, tag="wdwc", bufs=CT, name=f"wdwc{k}")
                wdma.dma_start(t[:], wdwc_d[k * 128:(k + 1) * 128, :])
                wdwc.append(t)
                t = wpool.tile([128, 1], F32, tag="bdwc", bufs=CT, name=f"bdwc{k}")
                wdma.dma_start(t[:], bdwc_d[k * 128:(k + 1) * 128, :])
                bdwc.append(t)
            bproj_row = wpool.tile([1, C], F32, tag="bprow")
            wdma.dma_start(bproj_row[:], bproj_d[:])
            bproj_bc = wpool.tile([128, C], F32, tag="bpbc")
            nc.gpsimd.partition_broadcast(bproj_bc[:], bproj_row[:])

            # persistent zero-bordered conv pad buffers (interior rewritten
            # per use; borders stay zero)
            vpad = []
            for i in range(2):
                t = wpool.tile([128, PAD * PAD], BF16, tag="vpad", bufs=2,
                               name=f"vpad{i}")
                nc.vector.memset(t[:], 0.0)
                vpad.append(t)
            # persistent v65 tiles (two sets of NCH); ones column written once
            v65_all = []
            for i in range(2 * NCH):
                t = wpool.tile([128, H * 65], BF16, tag="v65", bufs=2 * NCH,
                               name=f"v65_{i}")
                t3 = t[:].rearrange("p (h w) -> p h w", h=H, w=65)
                nc.vector.memset(t3[:, :, 64:65], 1.0)
                v65_all.append(t)

            import contextlib
            rep_ctx = tc.For_i(0, repeat, 1) if repeat > 1 else contextlib.nullcontext()
            with rep_ctx:
                state = {}
                evict_ctr = [0]
                evict_act_period = [2]  # 1-in-N evictions go to ACT

                def evict(dst_ap, src_ap):
                    # split PSUM->SBUF evictions between ACT and DVE
                    if evict_ctr[0] % evict_act_period[0] == 0:
                        nc.scalar.copy(dst_ap, src_ap)
                    else:
                        nc.vector.tensor_copy(dst_ap, src_ap)
                    evict_ctr[0] += 1

                mm_ctr = [0]
                mm_alt = [False]  # when True, alternate mm/sc pools

                def mm_tile():
                    mm_ctr[0] += 1
                    if mm_alt[0] and mm_ctr[0] % 2 == 0:
                        return psum_sc.tile([128, 768], F32, tag="sc",
                                            name="mmsc")
                    return psum_mm.tile([128, 768], F32, tag="mm", name="mmp")

                def emit_sample_inputs(s):
                    st = {"xT": [], "qk": [], "vch": [],
                          "v65": [v65_all[(s % 2) * NCH + ci] for ci in range(NCH)]}
                    if s == 0 and pre_x0 is not None:
                        st["xT"] = pre_x0
                        state[s] = st
                        return st
                    for k in range(CT):
                        t = work.tile([128, L], BF16, tag="xT", bufs=2 * CT,
                                      name=f"xT{k}")
                        nc.sync.dma_start(t[:], xT_d[s, k * 128:(k + 1) * 128, :])
                        st["xT"].append(t)
                    state[s] = st
                    return st

                def qkv_mtile(s, m):
                    st = state[s]
                    p = mm_tile()
                    for k in range(CT):
                        w_ap = wqkv[k][:, m * 128:(m + 1) * 128]
                        for (n0, nn) in ((0, 512), (512, 65)):
                            nc.tensor.matmul(
                                p[:, n0:n0 + nn], w_ap,
                                st["xT"][k][:, n0:n0 + nn],
                                start=(k == 0), stop=(k == CT - 1))
                    dst = work.tile([128, L], BF16,
                                    tag="qk" if m < 12 else "vch",
                                    bufs=24 if m < 12 else 2 * CT,
                                    name=f"qkv{m}")
                    evict(dst[:], p[:, 0:L])
                    (st["qk"] if m < 12 else st["vch"]).append(dst)

                def v65_chunk(s, ci):
                    st = state[s]
                    (l0, lp) = L_CHUNKS[ci]
                    t = st["v65"][ci]
                    t3 = t[:].rearrange("p (h w) -> p h w", h=H, w=65)
                    p = mm_tile()
                    for (n0, nn) in ((0, 512), (512, 256)):
                        for k in range(CT):
                            nc.tensor.matmul(
                                p[0:lp, n0:n0 + nn],
                                st["xT"][k][:, l0:l0 + lp],
                                wqkv[k][:, 2 * C + n0:2 * C + n0 + nn],
                                start=(k == 0), stop=(k == CT - 1))
                    evict(t3[0:lp, :, 0:64],
                          p[0:lp, 0:768].rearrange("p (h w) -> p h w", h=H, w=64))

                def make_fill_thunks(s):
                    return ([lambda m=m: qkv_mtile(s, m) for m in range(KT)]
                            + [lambda ci=ci: v65_chunk(s, ci) for ci in range(NCH)])

                def proj_chunk(s, ci):
                    st = state[s]
                    (l0, lp) = L_CHUNKS[ci]
                    attn = st["attn"]
                    p = mm_tile()
                    for (n0, nn) in ((0, 512), (512, 256)):
                        for k in range(CT):
                            nc.tensor.matmul(
                                p[0:lp, n0:n0 + nn],
                                attn[k][:, l0:l0 + lp],
                                wprojT[k][:, n0:n0 + nn],
                                start=(k == 0), stop=(k == CT - 1))
                    ysb = work.tile([128, C], F32, tag="ysb", bufs=2)
                    nc.vector.tensor_tensor(
                        out=ysb[0:lp, :], in0=p[0:lp, :], in1=bproj_bc[0:lp, :],
                        op=ADD)
                    nc.sync.dma_start(y_d[s, l0:l0 + lp, :], ysb[0:lp, :])

                def make_proj_thunks(s):
                    return [lambda ci=ci: proj_chunk(s, ci) for ci in range(NCH)]

                def scores_pair(s, hp):
                    # heads 2hp (rows 0:64) and 2hp+1 (rows 64:128) emitted
                    # chunk-interleaved: adjacent matmuls hit disjoint PE row
                    # groups and run concurrently on hardware
                    st = state[s]
                    qt = st["qk"][hp]
                    kt_ = st["qk"][6 + hp]
                    expA, expB = [], []
                    if OPT_PAIR:
                        order = [(l0, lp, hb, e) for (l0, lp) in L_CHUNKS
                                 for hb, e in ((0, expA), (64, expB))]
                    else:
                        order = ([(l0, lp, 0, expA) for (l0, lp) in L_CHUNKS]
                                 + [(l0, lp, 64, expB) for (l0, lp) in L_CHUNKS])
                    for (l0, lp, hb, exps) in order:
                        if True:
                            p = psum_sc.tile([128, 768], F32, tag="sc",
                                             name="scp")
                            for (n0, nn) in ((0, 512), (512, 65)):
                                nc.tensor.matmul(p[0:lp, n0:n0 + nn],
                                                 kt_[hb:hb + 64, l0:l0 + lp],
                                                 qt[hb:hb + 64, n0:n0 + nn],
                                                 start=True, stop=True)
                            e = work.tile([128, L], BF16, tag="expS", bufs=12,
                                          name="expSt")
                            nc.scalar.activation(e[0:lp, :], p[0:lp, 0:L], Exp,
                                                 scale=SCALE)
                            exps.append(e)
                    return expA, expB

                def pv_head(s, h, expS):
                    st = state[s]
                    pv = psum_pv.tile([128, L], F32, tag="pv")
                    for ci, (l0, lp) in enumerate(L_CHUNKS):
                        for (n0, nn) in ((0, 512), (512, 65)):
                            nc.tensor.matmul(
                                pv[0:65, n0:n0 + nn],
                                st["v65"][ci][0:lp, h * 65:(h + 1) * 65],
                                expS[ci][0:lp, n0:n0 + nn],
                                start=(ci == 0), stop=(ci == NCH - 1))
                    # NOTE: reciprocal_approx_fast needs a base-partition-0
                    # SBUF operand (PSUM or partition-offset reads return
                    # garbage on hardware), so stage the sums row via ACT
                    sums = work.tile([1, L], F32, tag="sums", bufs=2,
                                     name="sums")
                    nc.scalar.copy(sums[:], pv[64:65, :])
                    rec = work.tile([1, L], F32, tag="rec", bufs=2, name="rec")
                    nc.vector.reciprocal_approx_fast(out=rec[:], in_=sums[:])
                    bc = work.tile([64, L], F32, tag="bc", bufs=2, name="bc")
                    nc.gpsimd.partition_broadcast(bc[:], rec[:])
                    hb = (h % 2) * 64
                    nc.vector.tensor_tensor(
                        out=st["attn"][h // 2][hb:hb + 64, :],
                        in0=pv[0:64, :], in1=bc[:], op=MULT)

                def conv_prep(s, ct):
                    st = state[s]
                    vp = vpad[ct % 2]
                    vp3 = vp[:].rearrange("p (y x) -> p y x", y=PAD, x=PAD)
                    conv_eng = nc.gpsimd if OPT_POOLCONV else nc.vector
                    conv_eng.tensor_copy(
                        vp3[:, 1:1 + IMG, 1:1 + IMG],
                        st["vch"][ct][:, 1:L].rearrange("p (y x) -> p y x",
                                                        y=IMG, x=IMG))
                    acc = work.tile([128, IMG * IMG], BF16, tag="cacc", bufs=2,
                                    name="cacc")
                    acc3 = acc[:].rearrange("p (y x) -> p y x", y=IMG, x=IMG)

                    def tap(t):
                        return vp3[:, t // 3:t // 3 + IMG, t % 3:t % 3 + IMG]

                    nc.vector.tensor_scalar(
                        out=acc3, in0=tap(4), scalar1=wdwc[ct][:, 4:5],
                        scalar2=None, op0=MULT)
                    for i, t in enumerate([0, 1, 2, 3, 5, 6, 7, 8]):
                        tmp = work.tile([128, IMG * IMG], BF16, tag="ctmp",
                                        bufs=6, name="ctmp")
                        tmp3 = tmp[:].rearrange("p (y x) -> p y x", y=IMG, x=IMG)
                        nc.vector.tensor_scalar(
                            out=tmp3, in0=tap(t), scalar1=wdwc[ct][:, t:t + 1],
                            scalar2=None, op0=MULT)
                        # gpsimd absorbs part of the accumulation (DVE is the
                        # busier engine); it gets its own partial sum so the
                        # two engines' chains stay disjoint.
                        if OPT_POOLCONV and i == 2:
                            pacc = work.tile([128, IMG * IMG], BF16, tag="pacc",
                                             bufs=2, name="pacc")
                            nc.gpsimd.tensor_copy(pacc[:], tmp[:])
                        elif OPT_POOLCONV and i > 2:
                            nc.gpsimd.tensor_tensor(out=pacc[:], in0=pacc[:],
                                                    in1=tmp[:], op=ADD)
                        else:
                            nc.vector.tensor_tensor(out=acc[:], in0=acc[:],
                                                    in1=tmp[:], op=ADD)
                    if OPT_POOLCONV:
                        nc.vector.tensor_tensor(out=acc[:], in0=acc[:],
                                                in1=pacc[:], op=ADD)
                    return acc

                def conv_add(s, ct, acc):
                    # attn[:, 1:] += acc + b_dwc
                    nc.vector.scalar_tensor_tensor(
                        out=state[s]["attn"][ct][:, 1:L], in0=acc[:],
                        scalar=bdwc[ct][:, 0:1],
                        in1=state[s]["attn"][ct][:, 1:L],
                        op0=ADD, op1=ADD)

                # ---- prologue: sample 0 inputs + qkv/v65 emitted directly
                # (mm/sc pool alternation -- nothing else needs sc yet) ----
                emit_sample_inputs(0)
                mm_alt[0] = True
                for t in make_fill_thunks(0):
                    t()
                mm_alt[0] = False

                for s in range(S):
                    st = state[s]
                    st["attn"] = [work.tile([128, L], BF16, tag="attn", bufs=18,
                                            name=f"attn{ct}") for ct in range(CT)]
                    fillers = []
                    if s + 1 < S:
                        emit_sample_inputs(s + 1)
                        fillers += make_fill_thunks(s + 1)
                    # projections trail by two samples so the tail sample's
                    # exp-waits still have PE filler work
                    if s - 2 >= 0:
                        fillers += make_proj_thunks(s - 2)
                    if s == S - 1:
                        fillers += make_proj_thunks(s - 1)

                    if stages == "qkv":
                        zsrc = work.tile([128, L], F32, tag="zsrc", bufs=1,
                                         name="zsrc")
                        nc.vector.memset(zsrc[:], 0.0)
                        for ct in range(CT):
                            nc.vector.tensor_copy(st["attn"][ct][:], zsrc[:])
                        for t in fillers:
                            t()
                        continue

                    # ACT paces the attention inner loop (5 exp per head);
                    # keep it mostly exp there
                    evict_act_period[0] = 3
                    nf = len(fillers)
                    done = 0
                    for hp in range(H // 2):
                        expA, expB = scores_pair(s, hp)
                        if stages == "full":
                            acc = conv_prep(s, hp)
                        # PE fillers between scores and PV cover the exp wait
                        target = ((2 * hp + 1) * nf) // H
                        while done < target:
                            fillers[done]()
                            done += 1
                        pv_head(s, 2 * hp, expA)
                        target = ((2 * hp + 2) * nf) // H
                        while done < target:
                            fillers[done]()
                            done += 1
                        pv_head(s, 2 * hp + 1, expB)
                        if stages == "full":
                            conv_add(s, hp, acc)
                    evict_act_period[0] = 2

                # final projection (no attention loop left to hide it in)
                mm_alt[0] = True
                for t in make_proj_thunks(S - 1):
                    t()
                mm_alt[0] = False

    nc.compile()
    _CACHE[key] = nc
    return nc


def make_in_maps(x, w_qkv, w_proj, b_proj, w_dwc, b_dwc):
    x = np.asarray(x, dtype=np.float32)
    N = x.shape[0]
    assert N == N_CORES * S
    import ml_dtypes
    wqkvT = np.ascontiguousarray(
        np.asarray(w_qkv, np.float32).T.astype(ml_dtypes.bfloat16))    # [C, 3C]
    wprojT = np.ascontiguousarray(
        np.asarray(w_proj, np.float32).T.astype(ml_dtypes.bfloat16))   # [C, C]
    wdwc9 = np.ascontiguousarray(np.asarray(w_dwc, np.float32).reshape(C, 9))
    bdwc = np.ascontiguousarray(np.asarray(b_dwc, np.float32).reshape(C, 1))
    bproj = np.ascontiguousarray(np.asarray(b_proj, np.float32).reshape(1, C))

    in_maps = []
    for i in range(N_CORES):
        xs = x[i * S:(i + 1) * S]                       # [S, L, C]
        xT = np.ascontiguousarray(
            xs.transpose(0, 2, 1).astype(ml_dtypes.bfloat16))  # [S, C, L]
        in_maps.append({"xT": xT, "wqkvT": wqkvT, "wprojT": wprojT,
                        "wdwc": wdwc9, "bdwc": bdwc, "bproj": bproj})
    return in_maps


def kernel(x, w_qkv, w_proj, b_proj, w_dwc, b_dwc):
    global last_results
    nc = _build_nc()
    in_maps = make_in_maps(x, w_qkv, w_proj, b_proj, w_dwc, b_dwc)
    last_results = run_bass_kernel_spmd(nc, in_maps, list(range(N_CORES)))
    y = np.concatenate([r["y"] for r in last_results.results], axis=0)
    return y.astype(np.float32)


# revision 39
# speedup vs baseline: 1.1369x; 1.0137x over previous
"""Trainium2 Bass kernel for DeiT-style attention + depthwise-conv block.

Computes, for x [N=32, L=577, C=768]:
  qkv = x @ w_qkv.T -> q,k,v (12 heads, hd=64)
  attn = softmax(q k^T / 8) @ v
  out  = attn (+ depthwise3x3(v) on patch tokens) @ w_proj.T + b_proj

Sharding: data-parallel over batch, 4 samples per core x 8 NeuronCores.

Structure (per core): the attention inner loop is ACT(exp)-paced, so PE
work from the NEXT sample (qkv/v65 matmuls) and the TWO-samples-back
projection is interleaved as filler between each head's scores and PV
matmuls, keeping the tensor engine busy through the exp waits.  Head
pairs emit chunk-interleaved scores (disjoint PE row groups).  Weights
are SBUF-resident; sample-0 x is prefetched ahead of the weight loads.
Softmax normalize = ACT sums-copy -> DVE fast reciprocal -> gpsimd
partition-broadcast -> DVE multiply (reciprocal_approx_fast requires a
base-partition-0 SBUF input on hardware).  The depthwise conv runs on
the DVE as 9 tensor_scalar taps + tensor_tensor accumulates over
persistent zero-bordered pad buffers (gpsimd offload of these measured
2x slower on hardware despite a favorable cost model).
"""
import sys

sys.path.insert(0, "/opt/trn_rl_repo")

import numpy as np

import concourse.bacc as bacc
import concourse.mybir as mybir
import concourse.tile as tile
from concourse.bass_utils import run_bass_kernel_spmd

F32 = mybir.dt.float32
F32R = mybir.dt.float32r
BF16 = mybir.dt.bfloat16
Exp = mybir.ActivationFunctionType.Exp
MULT = mybir.AluOpType.mult
ADD = mybir.AluOpType.add
DIV = mybir.AluOpType.divide

N_CORES = 8
S = 4            # samples per core
C, L, H, HD = 768, 577, 12, 64
CT = C // 128    # 6 channel tiles
KT = 3 * C // 128  # 18 qkv row tiles
SCALE = HD ** -0.5
L_CHUNKS = [(i * 128, min(128, L - i * 128)) for i in range((L + 127) // 128)]
NCH = len(L_CHUNKS)  # 5
IMG = 24         # spatial side; L-1 == IMG*IMG
PAD = IMG + 2    # padded side

_CACHE = {}
last_results = None  # BassKernelResults of the most recent run (for test harness)

def _build_nc(repeat=1, stages="full"):
    key = (repeat, stages)
    if key in _CACHE:
        return _CACHE[key]
    nc = bacc.Bacc("TRN2", target_bir_lowering=False, debug=False,
                   num_devices=N_CORES)
    xT_d = nc.declare_dram_parameter("xT", [S, C, L], BF16, isOutput=False)
    wqkvT_d = nc.declare_dram_parameter("wqkvT", [C, 3 * C], BF16, isOutput=False)
    wprojT_d = nc.declare_dram_parameter("wprojT", [C, C], BF16, isOutput=False)
    wdwc_d = nc.declare_dram_parameter("wdwc", [C, 9], F32, isOutput=False)
    bdwc_d = nc.declare_dram_parameter("bdwc", [C, 1], F32, isOutput=False)
    bproj_d = nc.declare_dram_parameter("bproj", [1, C], F32, isOutput=False)
    y_d = nc.declare_dram_parameter("y", [S, L, C], F32, isOutput=True)

    with tile.TileContext(nc) as tc:
        with tc.tile_pool(name="wpool", bufs=1) as wpool, \
             tc.tile_pool(name="work", bufs=2) as work, \
             tc.tile_pool(name="mm", bufs=1, space="PSUM") as psum_mm, \
             tc.tile_pool(name="sc", bufs=2, space="PSUM") as psum_sc, \
             tc.tile_pool(name="pv", bufs=1, space="PSUM") as psum_pv:

            # ---- resident weights (loaded once; q parts first so the
            # first qkv matmuls can start before the rest arrives) ----
            wqkv = []
            for k in range(CT):
                t = wpool.tile([128, 3 * C], BF16, tag="wqkv", bufs=CT,
                               name=f"wqkv{k}")
                wqkv.append(t)
            # single-shot build: prefetch sample-0 x ahead of the weight
            # loads so the first qkv matmuls can start as early as possible
            pre_x0 = None
            if repeat == 1:
                pre_x0 = []
                for k in range(CT):
                    t = work.tile([128, L], BF16, tag="xT", bufs=2 * CT,
                                  name=f"xT0p{k}")
                    nc.sync.dma_start(t[:], xT_d[0, k * 128:(k + 1) * 128, :])
                    pre_x0.append(t)

            wdma = nc.sync
            for part in range(3):
                for k in range(CT):
                    wdma.dma_start(
                        wqkv[k][:, part * C:(part + 1) * C],
                        wqkvT_d[k * 128:(k + 1) * 128, part * C:(part + 1) * C])
            wprojT = []
            for k in range(CT):
                t = wpool.tile([128, C], BF16, tag="wprojT", bufs=CT,
                               name=f"wprojT{k}")
                wdma.dma_start(t[:], wprojT_d[k * 128:(k + 1) * 128, :])
                wprojT.append(t)
            wdwc = []
            bdwc = []
            for k in range(CT):
                t = wpool.tile([128, 9], F32, tag="wdwc", bufs=CT, name=f"wdwc{k}")
                wdma.dma_start(t[:], wdwc_d[k * 128:(k + 1) * 128, :])
                wdwc.append(t)
                t = wpool.tile([128, 1], F32, tag="bdwc", bufs=CT, name=f"bdwc{k}")
                wdma.dma_start(t[:], bdwc_d[k * 128:(k + 1) * 128, :])
                bdwc.append(t)
            bproj_row = wpool.tile([1, C], F32, tag="bprow")
            wdma.dma_start(bproj_row[:], bproj_d[:])
            bproj_bc = wpool.tile([128, C], F32, tag="bpbc")
            nc.gpsimd.partition_broadcast(bproj_bc[:], bproj_row[:])

            # persistent zero-bordered conv pad buffers (interior rewritten
            # per use; borders stay zero)
            vpad = []
            for i in range(2):
                t = wpool.tile([128, PAD * PAD], BF16, tag="vpad", bufs=2,
                               name=f"vpad{i}")
                nc.vector.memset(t[:], 0.0)
                vpad.append(t)
            # persistent v65 tiles (two sets of NCH); ones column written once
            v65_all = []
            for i in range(2 * NCH):
                t = wpool.tile([128, H * 65], BF16, tag="v65", bufs=2 * NCH,
                               name=f"v65_{i}")
                t3 = t[:].rearrange("p (h w) -> p h w", h=H, w=65)
                nc.vector.memset(t3[:, :, 64:65], 1.0)
                v65_all.append(t)

            import contextlib
            rep_ctx = tc.For_i(0, repeat, 1) if repeat > 1 else contextlib.nullcontext()
            with rep_ctx:
                state = {}
                evict_ctr = [0]
                evict_act_period = [2]  # 1-in-N evictions go to ACT

                def evict(dst_ap, src_ap):
                    # split PSUM->SBUF evictions between ACT and DVE
                    if evict_ctr[0] % evict_act_period[0] == 0:
                        nc.scalar.copy(dst_ap, src_ap)
                    else:
                        nc.vector.tensor_copy(dst_ap, src_ap)
                    evict_ctr[0] += 1

                mm_ctr = [0]
                mm_alt = [False]  # when True, alternate mm/sc pools

                def mm_tile():
                    mm_ctr[0] += 1
                    if mm_alt[0] and mm_ctr[0] % 2 == 0:
                        return psum_sc.tile([128, 768], F32, tag="sc",
                                            name="mmsc")
                    return psum_mm.tile([128, 768], F32, tag="mm", name="mmp")

                def emit_sample_inputs(s):
                    st = {"xT": [], "qk": [], "vch": [],
                          "v65": [v65_all[(s % 2) * NCH + ci] for ci in range(NCH)]}
                    if s == 0 and pre_x0 is not None:
                        st["xT"] = pre_x0
                        state[s] = st
                        return st
                    for k in range(CT):
                        t = work.tile([128, L], BF16, tag="xT", bufs=2 * CT,
                                      name=f"xT{k}")
                        nc.sync.dma_start(t[:], xT_d[s, k * 128:(k + 1) * 128, :])
                        st["xT"].append(t)
                    state[s] = st
                    return st

                def qkv_mtile(s, m):
                    st = state[s]
                    p = mm_tile()
                    for k in range(CT):
                        w_ap = wqkv[k][:, m * 128:(m + 1) * 128]
                        for (n0, nn) in ((0, 512), (512, 65)):
                            nc.tensor.matmul(
                                p[:, n0:n0 + nn], w_ap,
                                st["xT"][k][:, n0:n0 + nn],
                                start=(k == 0), stop=(k == CT - 1))
                    dst = work.tile([128, L], BF16,
                                    tag="qk" if m < 12 else "vch",
                                    bufs=24 if m < 12 else 2 * CT,
                                    name=f"qkv{m}")
                    evict(dst[:], p[:, 0:L])
                    (st["qk"] if m < 12 else st["vch"]).append(dst)

                def v65_chunk(s, ci):
                    st = state[s]
                    (l0, lp) = L_CHUNKS[ci]
                    t = st["v65"][ci]
                    t3 = t[:].rearrange("p (h w) -> p h w", h=H, w=65)
                    p = mm_tile()
                    for (n0, nn) in ((0, 512), (512, 256)):
                        for k in range(CT):
                            nc.tensor.matmul(
                                p[0:lp, n0:n0 + nn],
                                st["xT"][k][:, l0:l0 + lp],
                                wqkv[k][:, 2 * C + n0:2 * C + n0 + nn],
                                start=(k == 0), stop=(k == CT - 1))
                    evict(t3[0:lp, :, 0:64],
                          p[0:lp, 0:768].rearrange("p (h w) -> p h w", h=H, w=64))

                def make_fill_thunks(s):
                    return ([lambda m=m: qkv_mtile(s, m) for m in range(KT)]
                            + [lambda ci=ci: v65_chunk(s, ci) for ci in range(NCH)])

                def proj_chunk(s, ci):
                    st = state[s]
                    (l0, lp) = L_CHUNKS[ci]
                    attn = st["attn"]
                    p = mm_tile()
                    for (n0, nn) in ((0, 512), (512, 256)):
                        for k in range(CT):
                            nc.tensor.matmul(
                                p[0:lp, n0:n0 + nn],
                                attn[k][:, l0:l0 + lp],
                                wprojT[k][:, n0:n0 + nn],
                                start=(k == 0), stop=(k == CT - 1))
                    ysb = work.tile([128, C], F32, tag="ysb", bufs=2)
                    nc.vector.tensor_tensor(
                        out=ysb[0:lp, :], in0=p[0:lp, :], in1=bproj_bc[0:lp, :],
                        op=ADD)
                    nc.sync.dma_start(y_d[s, l0:l0 + lp, :], ysb[0:lp, :])

                def make_proj_thunks(s):
                    return [lambda ci=ci: proj_chunk(s, ci) for ci in range(NCH)]

                def scores_pair(s, hp):
                    # heads 2hp (rows 0:64) and 2hp+1 (rows 64:128) emitted
                    # chunk-interleaved: adjacent matmuls hit disjoint PE row
                    # groups and run concurrently on hardware
                    st = state[s]
                    qt = st["qk"][hp]
                    kt_ = st["qk"][6 + hp]
                    expA, expB = [], []
                    order = [(l0, lp, hb, e) for (l0, lp) in L_CHUNKS
                             for hb, e in ((0, expA), (64, expB))]
                    for (l0, lp, hb, exps) in order:
                        if True:
                            p = psum_sc.tile([128, 768], F32, tag="sc",
                                             name="scp")
                            for (n0, nn) in ((0, 512), (512, 65)):
                                nc.tensor.matmul(p[0:lp, n0:n0 + nn],
                                                 kt_[hb:hb + 64, l0:l0 + lp],
                                                 qt[hb:hb + 64, n0:n0 + nn],
                                                 start=True, stop=True)
                            e = work.tile([128, L], BF16, tag="expS", bufs=12,
                                          name="expSt")
                            nc.scalar.activation(e[0:lp, :], p[0:lp, 0:L], Exp,
                                                 scale=SCALE)
                            exps.append(e)
                    return expA, expB

                def pv_head(s, h, expS):
                    st = state[s]
                    pv = psum_pv.tile([128, L], F32, tag="pv")
                    for ci, (l0, lp) in enumerate(L_CHUNKS):
                        for (n0, nn) in ((0, 512), (512, 65)):
                            nc.tensor.matmul(
                                pv[0:65, n0:n0 + nn],
                                st["v65"][ci][0:lp, h * 65:(h + 1) * 65],
                                expS[ci][0:lp, n0:n0 + nn],
                                start=(ci == 0), stop=(ci == NCH - 1))
                    # NOTE: reciprocal_approx_fast needs a base-partition-0
                    # SBUF operand (PSUM or partition-offset reads return
                    # garbage on hardware), so stage the sums row via ACT
                    sums = work.tile([1, L], F32, tag="sums", bufs=2,
                                     name="sums")
                    nc.scalar.copy(sums[:], pv[64:65, :])
                    rec = work.tile([1, L], F32, tag="rec", bufs=2, name="rec")
                    nc.vector.reciprocal_approx_fast(out=rec[:], in_=sums[:])
                    bc = work.tile([64, L], F32, tag="bc", bufs=2, name="bc")
                    nc.gpsimd.partition_broadcast(bc[:], rec[:])
                    hb = (h % 2) * 64
                    nc.vector.tensor_tensor(
                        out=st["attn"][h // 2][hb:hb + 64, :],
                        in0=pv[0:64, :], in1=bc[:], op=MULT)

                def conv_prep(s, ct):
                    st = state[s]
                    vp = vpad[ct % 2]
                    vp3 = vp[:].rearrange("p (y x) -> p y x", y=PAD, x=PAD)
                    nc.vector.tensor_copy(
                        vp3[:, 1:1 + IMG, 1:1 + IMG],
                        st["vch"][ct][:, 1:L].rearrange("p (y x) -> p y x",
                                                        y=IMG, x=IMG))
                    acc = work.tile([128, IMG * IMG], BF16, tag="cacc", bufs=2,
                                    name="cacc")
                    acc3 = acc[:].rearrange("p (y x) -> p y x", y=IMG, x=IMG)

                    def tap(t):
                        return vp3[:, t // 3:t // 3 + IMG, t % 3:t % 3 + IMG]

                    nc.vector.tensor_scalar(
                        out=acc3, in0=tap(4), scalar1=wdwc[ct][:, 4:5],
                        scalar2=None, op0=MULT)
                    for t in [0, 1, 2, 3, 5, 6, 7, 8]:
                        tmp = work.tile([128, IMG * IMG], BF16, tag="ctmp",
                                        bufs=6, name="ctmp")
                        tmp3 = tmp[:].rearrange("p (y x) -> p y x", y=IMG, x=IMG)
                        nc.vector.tensor_scalar(
                            out=tmp3, in0=tap(t), scalar1=wdwc[ct][:, t:t + 1],
                            scalar2=None, op0=MULT)
                        nc.vector.tensor_tensor(out=acc[:], in0=acc[:],
                                                in1=tmp[:], op=ADD)
                    return acc

                def conv_add(s, ct, acc):
                    # attn[:, 1:] += acc + b_dwc
                    nc.vector.scalar_tensor_tensor(
                        out=state[s]["attn"][ct][:, 1:L], in0=acc[:],
                        scalar=bdwc[ct][:, 0:1],
                        in1=state[s]["attn"][ct][:, 1:L],
                        op0=ADD, op1=ADD)

                # ---- prologue: sample 0 inputs + qkv/v65 emitted directly
                # (mm/sc pool alternation -- nothing else needs sc yet) ----
                emit_sample_inputs(0)
                mm_alt[0] = True
                for t in make_fill_thunks(0):
                    t()
                mm_alt[0] = False

                for s in range(S):
                    st = state[s]
                    st["attn"] = [work.tile([128, L], BF16, tag="attn", bufs=18,
                                            name=f"attn{ct}") for ct in range(CT)]
                    fillers = []
                    if s + 1 < S:
                        emit_sample_inputs(s + 1)
                        fillers += make_fill_thunks(s + 1)
                    # projections trail by two samples so the tail sample's
                    # exp-waits still have PE filler work
                    if s - 2 >= 0:
                        fillers += make_proj_thunks(s - 2)
                    if s == S - 1:
                        fillers += make_proj_thunks(s - 1)

                    if stages == "qkv":
                        zsrc = work.tile([128, L], F32, tag="zsrc", bufs=1,
                                         name="zsrc")
                        nc.vector.memset(zsrc[:], 0.0)
                        for ct in range(CT):
                            nc.vector.tensor_copy(st["attn"][ct][:], zsrc[:])
                        for t in fillers:
                            t()
                        continue

                    # ACT paces the attention inner loop (5 exp per head);
                    # keep it mostly exp there
                    evict_act_period[0] = 3
                    nf = len(fillers)
                    done = 0
                    for hp in range(H // 2):
                        expA, expB = scores_pair(s, hp)
                        if stages == "full":
                            acc = conv_prep(s, hp)
                        # PE fillers between scores and PV cover the exp wait
                        target = ((2 * hp + 1) * nf) // H
                        while done < target:
                            fillers[done]()
                            done += 1
                        pv_head(s, 2 * hp, expA)
                        target = ((2 * hp + 2) * nf) // H
                        while done < target:
                            fillers[done]()
                            done += 1
                        pv_head(s, 2 * hp + 1, expB)
                        if stages == "full":
                            conv_add(s, hp, acc)
                    evict_act_period[0] = 2

                # final projection (no attention loop left to hide it in)
                mm_alt[0] = True
                for t in make_proj_thunks(S - 1):
                    t()
                mm_alt[0] = False

    nc.compile()
    _CACHE[key] = nc
    return nc


def make_in_maps(x, w_qkv, w_proj, b_proj, w_dwc, b_dwc):
    x = np.asarray(x, dtype=np.float32)
    N = x.shape[0]
    assert N == N_CORES * S
    import ml_dtypes
    wqkvT = np.ascontiguousarray(
        np.asarray(w_qkv, np.float32).T.astype(ml_dtypes.bfloat16))    # [C, 3C]
    wprojT = np.ascontiguousarray(
        np.asarray(w_proj, np.float32).T.astype(ml_dtypes.bfloat16))   # [C, C]
    wdwc9 = np.ascontiguousarray(np.asarray(w_dwc, np.float32).reshape(C, 9))
    bdwc = np.ascontiguousarray(np.asarray(b_dwc, np.float32).reshape(C, 1))
    bproj = np.ascontiguousarray(np.asarray(b_proj, np.float32).reshape(1, C))

    in_maps = []
    for i in range(N_CORES):
        xs = x[i * S:(i + 1) * S]                       # [S, L, C]
        xT = np.ascontiguousarray(
            xs.transpose(0, 2, 1).astype(ml_dtypes.bfloat16))  # [S, C, L]
        in_maps.append({"xT": xT, "wqkvT": wqkvT, "wprojT": wprojT,
                        "wdwc": wdwc9, "bdwc": bdwc, "bproj": bproj})
    return in_maps


def kernel(x, w_qkv, w_proj, b_proj, w_dwc, b_dwc):
    global last_results
    nc = _build_nc()
    in_maps = make_in_maps(x, w_qkv, w_proj, b_proj, w_dwc, b_dwc)
    last_results = run_bass_kernel_spmd(nc, in_maps, list(range(N_CORES)))
    y = np.concatenate([r["y"] for r in last_results.results], axis=0)
    return y.astype(np.float32)
